# revision 19
# baseline (speedup 1.0000x reference)
"""Trainium2 Bass kernel for nn_DecoderLayer_68461778698665 (segment_reduce).

reference:
    pooled = vmap(segment_sum)(nodes, node_graph_idx)   # [B, G, D]
    z = concat([pooled, global_latent], -1)             # [B, G, 2D]
    logits = z @ W + b                                  # [B, G, 1]

Shapes: B=16 packs, N=16384 nodes/pack, D=128, G=16 graphs/pack.

Strategy (data-parallel, 2 packs per core across 8 cores):
  - segment-sum as one-hot matmul on the TensorEngine: for each tile of
    128 nodes, onehot[n,g] = (idx[n] == g) built on the VectorEngine,
    then psum[128d,16g] += nodes_tile[128n,128d].T @ onehot[128n,16g]
    accumulated over all 128 tiles of a pack in one PSUM bank.
  - epilogue: logits^T[1,16] = Wt.T @ pooled^T + Wb.T @ glob^T + b
    (two tiny matmuls into one PSUM accumulation group).
  - host side only does sharding / layout prep: W split into halves,
    global transposed, idx laid out to match the node tile layout.
"""

import sys

sys.path.insert(0, "/opt/trn_rl_repo")

import numpy as np

import concourse.tile as tile
from concourse import bacc, bass, mybir
from concourse.bass_utils import run_bass_kernel_spmd

P = 128  # partitions
B, N, D, G = 16, 16384, 128, 16
NCORES = 8
B_LOC = B // NCORES  # packs per core
NODES_PER_CHUNK = 4096  # 2 MiB per DMA
J_PER_CHUNK = NODES_PER_CHUNK // P  # 32 node-tiles per chunk
F32 = mybir.dt.float32


def build_bass(
    b_loc: int = B_LOC,
    n_nodes: int = N,
    repeat: int = 1,
    hw_loop: int = 0,
    mode: str = "full",  # "full" | "dma" (skip PE/DVE) | "pe" (1 chunk DMA)
) -> bass.Bass:
    """One SPMD program; every core runs it on its own 2-pack shard.

    repeat>1 unrolls the whole body R times; hw_loop>0 wraps the body in a
    hardware For_i loop (both benchmarking only: they scale device time up
    so per-iteration HW time can be extracted from wall-clock diffs).
    """
    n_chunks = n_nodes // NODES_PER_CHUNK
    n_tiles = n_nodes // P  # node-tiles per pack

    # Bacc (not plain Bass): its compile() runs move_matmul_waits_to_ldweights
    # + generate_event_semaphores, which legalize Tile's multi-wait sync_infos
    # down to the 1-wait-per-instruction walrus limit.
    nc = bacc.Bacc()
    nodes_d = nc.dram_tensor("nodes", [b_loc, n_nodes, D], F32, kind="ExternalInput")
    # idxq[p][q, c*J + j] = idx[p, c*NODES_PER_CHUNK + q*J_PER_CHUNK + j] as f32,
    # with G extra iota columns (idxq[p][q, n_tiles+g] = g) appended so the
    # onehot TensorTensor depends on exactly one DMA (walrus caps TT at one
    # sync wait).
    idxq_d = nc.dram_tensor("idxq", [b_loc, P, n_tiles + G], F32, kind="ExternalInput")
    globt_d = nc.dram_tensor("globt", [b_loc, D, G], F32, kind="ExternalInput")
    wt_d = nc.dram_tensor("wt", [D, 1], F32, kind="ExternalInput")
    wb_d = nc.dram_tensor("wb", [D, 1], F32, kind="ExternalInput")
    bias_d = nc.dram_tensor("bias", [1, 1], F32, kind="ExternalInput")
    out_d = nc.dram_tensor("out", [b_loc, G], F32, kind="ExternalOutput")

    n_onehot_bufs = b_loc * n_chunks  # never recycle: keeps TT waits <= 1

    with tile.TileContext(nc) as tc:
        with (
            tc.tile_pool(name="const", bufs=1) as const_pool,
            tc.tile_pool(name="idx", bufs=2) as idx_pool,
            tc.tile_pool(name="glob", bufs=2) as glob_pool,
            tc.tile_pool(name="nodes", bufs=4) as nodes_pool,
            tc.tile_pool(name="onehot", bufs=n_onehot_bufs) as onehot_pool,
            tc.tile_pool(name="pooled", bufs=2) as pooled_pool,
            tc.tile_pool(name="outs", bufs=2) as out_pool,
            tc.tile_pool(name="ppsum", bufs=2, space="PSUM") as ppsum_pool,
            tc.tile_pool(name="lgpsum", bufs=2, space="PSUM") as lg_pool,
        ):
            wt_sb = const_pool.tile([D, 1], F32)
            wb_sb = const_pool.tile([D, 1], F32)
            bias_sb = const_pool.tile([1, 1], F32)
            nc.sync.dma_start(out=wt_sb[:], in_=wt_d[:])
            nc.sync.dma_start(out=wb_sb[:], in_=wb_d[:])
            nc.sync.dma_start(out=bias_sb[:], in_=bias_d[:])

            def emit_body():
                for p in [pp % b_loc for pp in range(b_loc * repeat)]:
                    emit_pack(p)

            def emit_pack(p):
                idxq_sb = idx_pool.tile([P, n_tiles + G], F32)
                globt_sb = glob_pool.tile([D, G], F32)
                nc.sync.dma_start(out=idxq_sb[:], in_=idxq_d[p])
                nc.sync.dma_start(out=globt_sb[:], in_=globt_d[p])

                ppsum = ppsum_pool.tile([P, G], F32)
                last_nodes_sb = None
                for c in range(n_chunks):
                    if mode == "pe" and c > 0:
                        nodes_sb = last_nodes_sb
                    else:
                        # node n = c*NODES_PER_CHUNK + q*J_PER_CHUNK + j lands
                        # at [partition q, free j*D:(j+1)*D] -> 16 KiB
                        # contiguous per partition, one 2 MiB contiguous DMA.
                        nodes_sb = nodes_pool.tile([P, J_PER_CHUNK * D], F32)
                        src = nodes_d[
                            p, c * NODES_PER_CHUNK : (c + 1) * NODES_PER_CHUNK, :
                        ].rearrange("(q j) d -> q (j d)", q=P)
                        nc.sync.dma_start(out=nodes_sb[:], in_=src)
                        last_nodes_sb = nodes_sb

                    if mode == "dma":
                        continue

                    onehot_sb = onehot_pool.tile([P, J_PER_CHUNK, G], F32)
                    nc.vector.tensor_tensor(
                        out=onehot_sb[:],
                        in0=idxq_sb[
                            :, c * J_PER_CHUNK : (c + 1) * J_PER_CHUNK, None
                        ].to_broadcast([P, J_PER_CHUNK, G]),
                        in1=idxq_sb[:, n_tiles : n_tiles + G][:, None, :].to_broadcast(
                            [P, J_PER_CHUNK, G]
                        ),
                        op=mybir.AluOpType.is_equal,
                    )

                    for j in range(J_PER_CHUNK):
                        nc.tensor.matmul(
                            out=ppsum[:],
                            lhsT=nodes_sb[:, j * D : (j + 1) * D],
                            rhs=onehot_sb[:, j, :],
                            start=(c == 0 and j == 0),
                            stop=(c == n_chunks - 1 and j == J_PER_CHUNK - 1),
                        )

                # pooled^T [d, g] -> logits^T [1, g]
                pooled_sb = pooled_pool.tile([P, G], F32)
                if mode == "dma":
                    nc.vector.tensor_copy(
                        out=pooled_sb[:], in_=last_nodes_sb[:, 0:G]
                    )
                else:
                    nc.vector.tensor_copy(out=pooled_sb[:], in_=ppsum[:])
                lg = lg_pool.tile([1, G], F32)
                nc.tensor.matmul(
                    out=lg[:], lhsT=wt_sb[:], rhs=pooled_sb[:], start=True, stop=False
                )
                nc.tensor.matmul(
                    out=lg[:], lhsT=wb_sb[:], rhs=globt_sb[:], start=False, stop=True
                )
                out_sb = out_pool.tile([1, G], F32)
                nc.vector.tensor_scalar_add(out_sb[:], lg[:], bias_sb[0:1, 0:1])
                nc.sync.dma_start(out=out_d[p : p + 1, :], in_=out_sb[:])

            if hw_loop > 0:
                with tc.For_i(
                    0, hw_loop, 1, hint_engines=(mybir.EngineType.PE,)
                ) as _i:
                    emit_body()
            else:
                emit_body()

    nc.compile()
    return nc


def _prep_shards(nodes, global_latent, W, b, node_graph_idx):
    """Host-side layout prep + sharding. Returns per-core input maps."""
    nodes = np.ascontiguousarray(nodes, dtype=np.float32)
    n_tiles = N // P
    # idxq[p][q, c*J+j] = idx[p, c*CHUNK + q*J + j]
    idxq = (
        node_graph_idx.reshape(B, N // NODES_PER_CHUNK, P, J_PER_CHUNK)
        .transpose(0, 2, 1, 3)
        .reshape(B, P, n_tiles)
        .astype(np.float32)
    )
    iota = np.broadcast_to(np.arange(G, dtype=np.float32), (B, P, G))
    idxq = np.ascontiguousarray(np.concatenate([idxq, iota], axis=2))
    globt = np.ascontiguousarray(
        np.asarray(global_latent, dtype=np.float32).transpose(0, 2, 1)
    )
    W = np.asarray(W, dtype=np.float32)
    wt = np.ascontiguousarray(W[:D, :])
    wb = np.ascontiguousarray(W[D:, :])
    bias = np.asarray(b, dtype=np.float32).reshape(1, 1)
    in_maps = []
    for i in range(NCORES):
        s = slice(i * B_LOC, (i + 1) * B_LOC)
        in_maps.append(
            {
                "nodes": nodes[s],
                "idxq": idxq[s],
                "globt": globt[s],
                "wt": wt,
                "wb": wb,
                "bias": bias,
            }
        )
    return in_maps


_CACHED_NC = None


def _get_nc():
    global _CACHED_NC
    if _CACHED_NC is None:
        _CACHED_NC = build_bass()
    return _CACHED_NC


def run_spmd(in_maps, **kwargs):
    nc = _get_nc()
    return run_bass_kernel_spmd(nc, in_maps, list(range(NCORES)), **kwargs)


def kernel(nodes, global_latent, W, b, node_graph_idx):
    in_maps = _prep_shards(nodes, global_latent, W, b, node_graph_idx)
    res = run_spmd(in_maps)
    out = np.concatenate([res.results[i]["out"] for i in range(NCORES)], axis=0)
    return out.reshape(B, G, 1).astype(np.float32)


# revision 24
# speedup vs baseline: 3.8118x; 3.8118x over previous
"""Trainium2 Bass kernel for nn_DecoderLayer_68461778698665 (segment_reduce).

reference:
    pooled = vmap(segment_sum)(nodes, node_graph_idx)   # [B, G, D]
    z = concat([pooled, global_latent], -1)             # [B, G, 2D]
    logits = z @ W + b                                  # [B, G, 1]

Shapes: B=16 packs, N=16384 nodes/pack, D=128, G=16 graphs/pack.

Strategy (data-parallel, 2 packs per core across 8 cores):
  - segment-sum as one-hot matmul on the TensorEngine: for each tile of
    128 nodes, onehot[n,g] = (idx[n] == g) built on the VectorEngine,
    then psum[128d,16g] += nodes_tile[128n,128d].T @ onehot[128n,16g]
    accumulated over all 128 tiles of a pack in one PSUM bank.
  - epilogue: logits^T[1,16] = Wt.T @ pooled^T + Wb.T @ glob^T + b
    (two tiny matmuls into one PSUM accumulation group).
  - host side only does sharding / layout prep: W split into halves,
    global transposed, idx laid out to match the node tile layout.
"""

import sys

sys.path.insert(0, "/opt/trn_rl_repo")

import numpy as np

import concourse.tile as tile
from concourse import bacc, bass, mybir
from concourse.bass_utils import run_bass_kernel_spmd

P = 128  # partitions
B, N, D, G = 16, 16384, 128, 16
NCORES = 8
B_LOC = B // NCORES  # packs per core
NODES_PER_CHUNK = 8192  # 4 MiB per DMA
J_PER_CHUNK = NODES_PER_CHUNK // P  # 64 node-tiles per chunk
F32 = mybir.dt.float32


def build_bass(
    b_loc: int = B_LOC,
    n_nodes: int = N,
    repeat: int = 1,
    hw_loop: int = 0,
    mode: str = "full",  # "full" | "dma" (skip PE/DVE) | "pe" (1 chunk DMA)
) -> bass.Bass:
    """One SPMD program; every core runs it on its own 2-pack shard.

    repeat>1 unrolls the whole body R times; hw_loop>0 wraps the body in a
    hardware For_i loop (both benchmarking only: they scale device time up
    so per-iteration HW time can be extracted from wall-clock diffs).
    """
    n_chunks = n_nodes // NODES_PER_CHUNK
    n_tiles = n_nodes // P  # node-tiles per pack

    # Bacc (not plain Bass): its compile() runs move_matmul_waits_to_ldweights
    # + generate_event_semaphores, which legalize Tile's multi-wait sync_infos
    # down to the 1-wait-per-instruction walrus limit.
    nc = bacc.Bacc()
    nodes_d = nc.dram_tensor("nodes", [b_loc, n_nodes, D], F32, kind="ExternalInput")
    # idxq[p][q, c*J + j] = idx[p, c*NODES_PER_CHUNK + q*J_PER_CHUNK + j] as f32,
    # with G extra iota columns (idxq[p][q, n_tiles+g] = g) appended so the
    # onehot TensorTensor depends on exactly one DMA (walrus caps TT at one
    # sync wait).
    idxq_d = nc.dram_tensor("idxq", [b_loc, P, n_tiles + G], F32, kind="ExternalInput")
    globt_d = nc.dram_tensor("globt", [b_loc, D, G], F32, kind="ExternalInput")
    wt_d = nc.dram_tensor("wt", [D, 1], F32, kind="ExternalInput")
    wb_d = nc.dram_tensor("wb", [D, 1], F32, kind="ExternalInput")
    bias_d = nc.dram_tensor("bias", [1, 1], F32, kind="ExternalInput")
    out_d = nc.dram_tensor("out", [b_loc, G], F32, kind="ExternalOutput")

    n_onehot_bufs = b_loc * n_chunks  # never recycle: keeps TT waits <= 1

    with tile.TileContext(nc) as tc:
        with (
            tc.tile_pool(name="const", bufs=1) as const_pool,
            tc.tile_pool(name="idx", bufs=2) as idx_pool,
            tc.tile_pool(name="glob", bufs=2) as glob_pool,
            tc.tile_pool(name="nodes", bufs=3) as nodes_pool,
            tc.tile_pool(name="onehot", bufs=n_onehot_bufs) as onehot_pool,
            tc.tile_pool(name="pooled", bufs=2) as pooled_pool,
            tc.tile_pool(name="outs", bufs=2) as out_pool,
            tc.tile_pool(name="ppsum", bufs=2, space="PSUM") as ppsum_pool,
            tc.tile_pool(name="lgpsum", bufs=2, space="PSUM") as lg_pool,
        ):
            wt_sb = const_pool.tile([D, 1], F32)
            wb_sb = const_pool.tile([D, 1], F32)
            bias_sb = const_pool.tile([1, 1], F32)
            nc.scalar.dma_start(out=wt_sb[:], in_=wt_d[:])
            nc.scalar.dma_start(out=wb_sb[:], in_=wb_d[:])
            nc.scalar.dma_start(out=bias_sb[:], in_=bias_d[:])

            def emit_body():
                for p in [pp % b_loc for pp in range(b_loc * repeat)]:
                    emit_pack(p)

            def emit_pack(p):
                idxq_sb = idx_pool.tile([P, n_tiles + G], F32)
                globt_sb = glob_pool.tile([D, G], F32)
                nc.sync.dma_start(out=idxq_sb[:], in_=idxq_d[p])
                nc.scalar.dma_start(out=globt_sb[:], in_=globt_d[p])

                ppsum = ppsum_pool.tile([P, G], F32)
                last_nodes_sb = None
                for c in range(n_chunks):
                    if mode == "pe" and c > 0:
                        nodes_sb = last_nodes_sb
                    else:
                        # node n = c*NODES_PER_CHUNK + q*J_PER_CHUNK + j lands
                        # at [partition q, free j*D:(j+1)*D] -> 16 KiB
                        # contiguous per partition, one 2 MiB contiguous DMA.
                        nodes_sb = nodes_pool.tile([P, J_PER_CHUNK * D], F32)
                        src = nodes_d[
                            p, c * NODES_PER_CHUNK : (c + 1) * NODES_PER_CHUNK, :
                        ].rearrange("(q j) d -> q (j d)", q=P)
                        # alternate the two HWDGE rings (SP / ACT) so the
                        # per-DMA fixed costs overlap across rings
                        dma_eng = nc.sync if (p * n_chunks + c) % 2 == 0 else nc.scalar
                        dma_eng.dma_start(out=nodes_sb[:], in_=src)
                        last_nodes_sb = nodes_sb

                    if mode == "dma":
                        continue

                    onehot_sb = onehot_pool.tile([P, J_PER_CHUNK, G], F32)
                    nc.vector.tensor_tensor(
                        out=onehot_sb[:],
                        in0=idxq_sb[
                            :, c * J_PER_CHUNK : (c + 1) * J_PER_CHUNK, None
                        ].to_broadcast([P, J_PER_CHUNK, G]),
                        in1=idxq_sb[:, n_tiles : n_tiles + G][:, None, :].to_broadcast(
                            [P, J_PER_CHUNK, G]
                        ),
                        op=mybir.AluOpType.is_equal,
                    )

                    for j in range(J_PER_CHUNK):
                        nc.tensor.matmul(
                            out=ppsum[:],
                            lhsT=nodes_sb[:, j * D : (j + 1) * D],
                            rhs=onehot_sb[:, j, :],
                            start=(c == 0 and j == 0),
                            stop=(c == n_chunks - 1 and j == J_PER_CHUNK - 1),
                        )

                # pooled^T [d, g] -> logits^T [1, g]
                pooled_sb = pooled_pool.tile([P, G], F32)
                if mode == "dma":
                    nc.vector.tensor_copy(
                        out=pooled_sb[:], in_=last_nodes_sb[:, 0:G]
                    )
                else:
                    nc.vector.tensor_copy(out=pooled_sb[:], in_=ppsum[:])
                lg = lg_pool.tile([1, G], F32)
                nc.tensor.matmul(
                    out=lg[:], lhsT=wt_sb[:], rhs=pooled_sb[:], start=True, stop=False
                )
                nc.tensor.matmul(
                    out=lg[:], lhsT=wb_sb[:], rhs=globt_sb[:], start=False, stop=True
                )
                out_sb = out_pool.tile([1, G], F32)
                nc.vector.tensor_scalar_add(out_sb[:], lg[:], bias_sb[0:1, 0:1])
                nc.sync.dma_start(out=out_d[p : p + 1, :], in_=out_sb[:])

            if hw_loop > 0:
                with tc.For_i(
                    0, hw_loop, 1, hint_engines=(mybir.EngineType.PE,)
                ) as _i:
                    emit_body()
            else:
                emit_body()

    nc.compile()
    return nc


def _prep_shards(nodes, global_latent, W, b, node_graph_idx):
    """Host-side layout prep + sharding. Returns per-core input maps."""
    nodes = np.ascontiguousarray(nodes, dtype=np.float32)
    n_tiles = N // P
    # idxq[p][q, c*J+j] = idx[p, c*CHUNK + q*J + j]
    idxq = (
        node_graph_idx.reshape(B, N // NODES_PER_CHUNK, P, J_PER_CHUNK)
        .transpose(0, 2, 1, 3)
        .reshape(B, P, n_tiles)
        .astype(np.float32)
    )
    iota = np.broadcast_to(np.arange(G, dtype=np.float32), (B, P, G))
    idxq = np.ascontiguousarray(np.concatenate([idxq, iota], axis=2))
    globt = np.ascontiguousarray(
        np.asarray(global_latent, dtype=np.float32).transpose(0, 2, 1)
    )
    W = np.asarray(W, dtype=np.float32)
    wt = np.ascontiguousarray(W[:D, :])
    wb = np.ascontiguousarray(W[D:, :])
    bias = np.asarray(b, dtype=np.float32).reshape(1, 1)
    in_maps = []
    for i in range(NCORES):
        s = slice(i * B_LOC, (i + 1) * B_LOC)
        in_maps.append(
            {
                "nodes": nodes[s],
                "idxq": idxq[s],
                "globt": globt[s],
                "wt": wt,
                "wb": wb,
                "bias": bias,
            }
        )
    return in_maps


_CACHED_NC = None


def _get_nc():
    global _CACHED_NC
    if _CACHED_NC is None:
        _CACHED_NC = build_bass()
    return _CACHED_NC


def run_spmd(in_maps, **kwargs):
    nc = _get_nc()
    return run_bass_kernel_spmd(nc, in_maps, list(range(NCORES)), **kwargs)


def kernel(nodes, global_latent, W, b, node_graph_idx):
    in_maps = _prep_shards(nodes, global_latent, W, b, node_graph_idx)
    res = run_spmd(in_maps)
    out = np.concatenate([res.results[i]["out"] for i in range(NCORES)], axis=0)
    return out.reshape(B, G, 1).astype(np.float32)
